# revision 20
# baseline (speedup 1.0000x reference)
"""Trainium2 Bass kernel for nn_AttentionTail.

Reference (B=2, N=300, C=256, H=2 heads, hd=128, L=21760):
  q = query @ Wq.T ; k = key @ Wk.T   (2 heads of 128)
  attn[b,n,l,h] = scale * <q_h, k_h>
  per level i (128^2, 64^2, 32^2, 16^2 keys): z = relu(attn_i @ Wl[i].T + bl[i])
  bilinear-upsample each level map to 128x128, concat channels,
  mask = relu(concat @ Wf.T + bf)

Host folds Wq, Wk, Wl, Wf, scale into 8 per-(level,channel) query vectors:
  qhat_{i,c} = scale*Wf[0,2i+c] * [Wl[i][c,0]*qp[:,:128] | Wl[i][c,1]*qp[:,128:]] @ Wk
  v_{i,c}[n,l] = <qhat_{i,c}[n], key[l]> ;  b_{i,c} = Wf[0,2i+c]*bl[i][c]
  contribution = relu(v+b) if Wf>0 else min(v+b, 0)   (= Wf * relu(attn-path))
  y_i = contrib_0 + contrib_1
  mask = relu(y_0 + sum_{i>=1} Ur_i @ Y_i @ Uc_i.T + bf)

Device (8 cores SPMD, B x N-quarter sharding, 75 queries/core):
  stream keyT fp16 (host-transposed + C-halves packed per span into one
  [128, 1024] tile = 2KB DMA lines, 6-deep prefetch), fp16 matmuls
  (N=150 used cols), levels 1-3 streamed first so their upsample tail
  overlaps the level-0 stream. Per span: one ACT relu for channel 1,
  then one fused scalar_tensor_tensor per target slab
  ((v0 max/min 0) +/- u1) writing y0 / s_l layouts directly (channel-0
  relu + combine + scatter in a single DVE pass; per-level combines
  merged across key blocks via strided 3D APs). PE-matmul separable
  upsample, PE transposes, final relu, fp16 transposed store (host
  upconverts to fp32). gpsimd is unusable here: no PSUM port, and every
  elementwise op in this kernel sources PSUM.
"""

import sys
import numpy as np

sys.path.insert(0, "/opt/trn_rl_repo")

import concourse.bass as bass
import concourse.bacc as bacc
import concourse.tile as tile
from concourse import mybir
from concourse.bass_utils import run_bass_kernel_spmd

F32 = mybir.dt.float32
F16 = mybir.dt.float16

B, N, C = 2, 300, 256
HD = 128
SCALE = HD ** -0.5
HW_LVL = ((128, 128), (64, 64), (32, 32), (16, 16))
L = 21760
L_DEV = 22016            # lvl3 padded h 16->32 on host (+256 zero cols)
LVL_OFF = [0, 16384, 20480, 21504]
NQ = 75
NCORES = 8

# levels 3,2,1 streamed first so their upsample tail overlaps the long
# level-0 stream; level 0 last.
SPANS_HI = ([(16384 + s * 512, 512, 1) for s in range(8)]
            + [(20480 + s * 512, 512, 2) for s in range(2)]
            + [(21504, 512, 3)])
SPANS_L0 = [(s * 512, 512, 0) for s in range(32)]

LVL_H = [128, 64, 32, 32]   # partition rows per block (lvl3 zero-padded)
LVL_W = [128, 64, 32, 16]
WSUB = [1, 2, 4, 4]          # w-columns per 128-key block
WBASE = [0, 0, 64, 96]       # row base of each level inside zcat / uc


def interp_matrix(src, dst):
    U = np.zeros((dst, src), np.float32)
    s = src / dst
    for d in range(dst):
        x = (d + 0.5) * s - 0.5
        x0 = int(np.floor(x))
        fr = x - x0
        a, b = max(0, min(src - 1, x0)), max(0, min(src - 1, x0 + 1))
        U[d, a] += 1 - fr
        U[d, b] += fr
    return U


def _build_program(signs, bf_val, reps=1, b0_zero=True):
    nc = bacc.Bacc("TRN2", target_bir_lowering=False)
    # keyT2: per 512-key span, both C-halves packed: [128, 2*L_DEV]
    keyT = nc.dram_tensor("keyT", [128, 2 * L_DEV], F16, kind="ExternalInput")
    qhatT = nc.dram_tensor("qhatT", [128, 1200], F16, kind="ExternalInput")
    bias_in = nc.dram_tensor("bias_in", [1, 9], F32, kind="ExternalInput")
    ur_in = nc.dram_tensor("ur_in", [64, 384], F16, kind="ExternalInput")
    uc_in = nc.dram_tensor("uc_in", [112, 128], F16, kind="ExternalInput")
    ident_in = nc.dram_tensor("ident_in", [128, 128], F16, kind="ExternalInput")
    out_d = nc.dram_tensor("out", [NQ, 128 * 128], F16, kind="ExternalOutput")

    with tile.TileContext(nc) as tc:
        for _ in range(reps):
            _body(nc, tc, keyT, qhatT, bias_in, ur_in, uc_in, ident_in,
                  out_d, signs, bf_val, b0_zero)
    nc.compile()
    return nc


def _body(nc, tc, keyT, qhatT, bias_in, ur_in, uc_in, ident_in,
          out_d, signs, bf_val, b0_zero):
    from contextlib import ExitStack
    ctx = ExitStack()
    with ctx:
        consts = ctx.enter_context(tc.tile_pool(name="consts", bufs=1))
        kpool = ctx.enter_context(tc.tile_pool(name="kpool", bufs=10))
        upool = ctx.enter_context(tc.tile_pool(name="upool", bufs=4))
        spool = ctx.enter_context(tc.tile_pool(name="spool", bufs=1))
        zpool = ctx.enter_context(tc.tile_pool(name="zpool", bufs=1))
        fpool = ctx.enter_context(tc.tile_pool(name="fpool", bufs=3))
        ps_attn = ctx.enter_context(tc.tile_pool(name="ps_attn", bufs=2, space="PSUM"))
        ps_up = ctx.enter_context(tc.tile_pool(name="ps_up", bufs=2, space="PSUM"))
        ps_tr = ctx.enter_context(tc.tile_pool(name="ps_tr", bufs=2, space="PSUM"))

        # ---- constants ----
        qh01 = consts.tile([128, 1200], F16, name="qh01")
        nc.sync.dma_start(out=qh01, in_=qhatT[:, :])
        bias_sb = consts.tile([128, 9], F32, name="bias_sb")
        nc.sync.dma_start(out=bias_sb, in_=bias_in[0:1, :].to_broadcast([128, 9]))
        ur = consts.tile([64, 384], F16, name="ur")
        uc = consts.tile([112, 128], F16, name="uc")
        ident = consts.tile([128, 128], F16, name="ident")

        # ---- level-map buffers ----
        y0 = spool.tile([128, NQ * 128], F16, name="y0")          # [c, (n, r)]
        sr_l = [None,
                spool.tile([64, 64 * NQ], F16, name="s1r"),       # [h, (w, n)]
                spool.tile([32, 32 * NQ], F16, name="s2r"),
                spool.tile([32, 16 * NQ], F16, name="s3r")]

        zs = [None,
              zpool.tile([128, 64 * NQ], F16, name="z1"),
              zpool.tile([128, 32 * NQ], F16, name="z2"),
              zpool.tile([128, 16 * NQ], F16, name="z3")]
        zcat = zpool.tile([112, NQ * 128], F16, name="zcat")

        def emit_span(off, ln, lvl):
            nblk = ln // 128
            k01 = kpool.tile([128, 1024], F16, tag="k01", name="k01")
            nc.sync.dma_start(out=k01, in_=keyT[:, 2 * off:2 * off + 1024])
            k0 = k01[:, 0:512]
            k1 = k01[:, 512:1024]
            ps = ps_attn.tile([128, 1024], F32, tag="ps", name="ps")
            for j in range(nblk):
                pslice = ps[:, j * 256:j * 256 + 150]
                nc.tensor.matmul(pslice,
                                 k0[:, j * 128:(j + 1) * 128],
                                 qh01[:, lvl * 150:(lvl + 1) * 150],
                                 start=True, stop=False)
                nc.tensor.matmul(pslice,
                                 k1[:, j * 128:(j + 1) * 128],
                                 qh01[:, 600 + lvl * 150:600 + (lvl + 1) * 150],
                                 start=False, stop=True)

            # ch1 -> u1 = relu(sign1*(v1 + b1)) on ACT (alternating gpsimd
            # for lvl0 to balance); ch0 fused into the combine STT:
            # dst = (v0 +b0 max/min 0) +/- u1.
            u1 = upool.tile([128, 300], F16, tag="u1", name="u1")
            psv = ps.rearrange("p (j x) -> p j x", x=256)
            s0, s1sn = signs[lvl][0], signs[lvl][1]
            src1 = psv[:, 0:nblk, NQ:2 * NQ]
            dst1 = u1.rearrange("p (j n) -> p j n", n=NQ)[:, 0:nblk, :]
            bval1 = bias_sb[:, lvl * 2 + 1:lvl * 2 + 2]
            use_gp = (not b0_zero and s1sn < 0)
            if use_gp:
                # (v1 + b1) max/min 0 keeps the sign; combine with add
                op2 = mybir.AluOpType.max if s1sn > 0 else mybir.AluOpType.min
                nc.vector.tensor_scalar(dst1, src1, bval1, 0.0,
                                        mybir.AluOpType.add, op2)
                comb_op1 = mybir.AluOpType.add
            else:
                # ACT relu with scale=sign1 gives |contribution|; combine
                # with add/subtract by sign.
                nc.scalar.activation(dst1, src1,
                                     mybir.ActivationFunctionType.Relu,
                                     bias=bval1, scale=float(s1sn))
                comb_op1 = (mybir.AluOpType.add if s1sn > 0
                            else mybir.AluOpType.subtract)
            op0 = mybir.AluOpType.max if s0 > 0 else mybir.AluOpType.min
            src0 = psv[:, 0:nblk, 0:NQ]
            u1v = dst1
            if not b0_zero:
                # general path: materialize ch0 contribution, then STT
                # degenerates to plain add of the two tiles.
                u0 = upool.tile([128, 300], F16, tag="u0", name="u0")
                d0 = u0.rearrange("p (j n) -> p j n", n=NQ)[:, 0:nblk, :]
                nc.vector.tensor_scalar(
                    d0, src0, bias_sb[:, lvl * 2:lvl * 2 + 1], 0.0,
                    mybir.AluOpType.add, op0)
                src0 = d0
                op0 = mybir.AluOpType.add       # (u0 + 0.0) comb u1

            if lvl == 0:
                r0 = off // 128
                dstv = y0.rearrange("p (n r) -> p n r", r=128)[:, :, r0:r0 + nblk].transpose([0, 2, 1])
                nc.vector.scalar_tensor_tensor(dstv, src0, 0.0, u1v,
                                               op0, comb_op1)
            else:
                # one STT per (wsub): dst s_l[h, (w,n)] cols w=(jb0+j)*ws+wsub
                h, ws = LVL_H[lvl], WSUB[lvl]
                jb0 = (off - LVL_OFF[lvl]) // 128
                eng = nc.vector
                for wsub in range(ws):
                    pa = wsub * h
                    dst = sr_l[lvl].rearrange("p (jj ww n) -> p jj ww n",
                                              ww=ws, n=NQ)[:, jb0:jb0 + nblk, wsub, :]
                    eng.scalar_tensor_tensor(
                        dst,
                        src0[pa:pa + h, :, :],
                        0.0,
                        u1[pa:pa + h, :].rearrange("p (j n) -> p j n", n=NQ)[:, 0:nblk, :],
                        op0, comb_op1)

        def emit_tail(lvl):
            # step A: Z_l[r, (w, n)] = Ur_l @ s_l
            h = LVL_H[lvl]
            tot = LVL_W[lvl] * NQ
            lhs = ur[0:h, (lvl - 1) * 128:lvl * 128]
            for ci in range((tot + 479) // 480):
                c0 = ci * 480
                cn = min(480, tot - c0)
                zp = ps_up.tile([128, 512], F32, tag="zp", name="zp")
                nc.tensor.matmul(zp[:, :cn], lhs,
                                 sr_l[lvl][:, c0:c0 + cn],
                                 start=True, stop=True)
                nc.vector.tensor_copy(zs[lvl][:, c0:c0 + cn], zp[:, :cn])
            # transpose Z per query (batched 4) -> zcat[w_cat, (n, r)]
            w = LVL_W[lvl]
            zv = zs[lvl].rearrange("p (w n) -> p w n", n=NQ)
            for g in range((NQ + 3) // 4):
                nsz = min(4, NQ - g * 4)
                pt = ps_tr.tile([128, 512], F16, tag="pt", name="pt")
                for k in range(nsz):
                    n = g * 4 + k
                    nc.tensor.matmul(pt[0:w, k * 128:(k + 1) * 128],
                                     zv[:, :, n],
                                     ident,
                                     is_transpose=True)
                nc.vector.tensor_copy(
                    zcat[WBASE[lvl]:WBASE[lvl] + w,
                         g * 512:g * 512 + nsz * 128],
                    pt[0:w, 0:nsz * 128])

        # levels 1,2,3 streamed + tailed first; long level-0 stream last
        for off, ln, lvl in SPANS_HI:
            emit_span(off, ln, lvl)
        # tail-only constants load behind the HI spans, ahead of the tails
        nc.sync.dma_start(out=ur, in_=ur_in[:, :])
        nc.sync.dma_start(out=uc, in_=uc_in[:, :])
        nc.sync.dma_start(out=ident, in_=ident_in[:, :])
        for lvl in (1, 2, 3):
            emit_tail(lvl)
        for off, ln, lvl in SPANS_L0:
            emit_span(off, ln, lvl)

        # ---- step B + level-0 add + final relu + transpose + store ----
        for g in range((NQ + 3) // 4):
            nsz = min(4, NQ - g * 4)
            nn = nsz * 128
            pb = ps_up.tile([128, 512], F32, tag="zp", name="pb")
            nc.tensor.matmul(pb[:, :nn], uc[:, :],
                             zcat[:, g * 512:g * 512 + nn],
                             start=True, stop=True)
            fin = fpool.tile([128, 512], F16, tag="fin", name="fin")
            nc.vector.tensor_tensor(fin[:, :nn], pb[:, :nn],
                                    y0[:, g * 512:g * 512 + nn],
                                    mybir.AluOpType.add)
            nc.scalar.activation(fin[:, :nn], fin[:, :nn],
                                 mybir.ActivationFunctionType.Relu,
                                 bias=bias_sb[:, 8:9], scale=1.0)
            pt2 = ps_tr.tile([128, 512], F16, tag="pt", name="pt2")
            for k in range(nsz):
                nc.tensor.matmul(pt2[:, k * 128:(k + 1) * 128],
                                 fin[:, k * 128:(k + 1) * 128],
                                 ident,
                                 is_transpose=True)
            outT = fpool.tile([128, 512], F16, tag="outT", name="outT")
            nc.vector.tensor_copy(outT[:, :nn], pt2[:, :nn])
            dram = out_d[g * 4:g * 4 + nsz, :].rearrange("n (r c) -> r n c", c=128)
            nc.sync.dma_start(out=dram,
                              in_=outT.rearrange("p (n c) -> p n c", c=128)[:, 0:nsz, :])


def _host_prep(query, key, Wq, Wk, Wl, bl, Wf, bf):
    query = np.asarray(query, np.float32)
    key = np.asarray(key, np.float32)
    Wq, Wk = np.asarray(Wq, np.float32), np.asarray(Wk, np.float32)
    Wl, bl = np.asarray(Wl, np.float32), np.asarray(bl, np.float32)
    Wf, bf = np.asarray(Wf, np.float32), np.asarray(bf, np.float32)

    qproj = query @ Wq.T
    qhat = np.zeros((4, 2, B, N, C), np.float32)
    biases = np.zeros((1, 9), np.float32)
    biases[0, 8] = float(bf[0])
    signs = [[1, 1] for _ in range(4)]
    for i in range(4):
        for c in range(2):
            wf = float(Wf[0, 2 * i + c])
            qt = np.concatenate([Wl[i][c, 0] * qproj[..., :HD],
                                 Wl[i][c, 1] * qproj[..., HD:]], -1)
            qhat[i, c] = (SCALE * wf) * (qt @ Wk)
            biases[0, i * 2 + c] = wf * bl[i][c]
            signs[i][c] = 1 if wf >= 0 else -1

    # keyT2 per batch: [128, 2*L_DEV]; per 512-key span both C-halves packed
    keyTs = []
    for b in range(B):
        cols = [key[b, :16384]]
        for i in (1, 2):
            h, w = HW_LVL[i]
            blk = key[b, LVL_OFF[i]:LVL_OFF[i] + h * w].reshape(h, w, C)
            cols.append(np.ascontiguousarray(blk.transpose(1, 0, 2)).reshape(-1, C))
        blk3 = key[b, LVL_OFF[3]:LVL_OFF[3] + 256].reshape(16, 16, C)
        blk3 = np.concatenate([blk3, np.zeros((16, 16, C), np.float32)], 0)  # h pad
        cols.append(np.ascontiguousarray(blk3.transpose(1, 0, 2)).reshape(-1, C))
        kb = np.concatenate(cols, 0)                       # [L_DEV, C]
        kt = kb.T.astype(np.float16)                       # [C, L_DEV]
        # pack spans: [128, 2*L_DEV], span s cols = [half0 512 | half1 512]
        k2 = np.zeros((128, 2 * L_DEV), np.float16)
        for s in range(L_DEV // 512):
            k2[:, s * 1024:s * 1024 + 512] = kt[0:128, s * 512:(s + 1) * 512]
            k2[:, s * 1024 + 512:s * 1024 + 1024] = kt[128:256, s * 512:(s + 1) * 512]
        keyTs.append(k2)

    ur_in = np.zeros((64, 384), np.float32)
    uc_in = np.zeros((112, 128), np.float32)
    for i in (1, 2, 3):
        h, w = HW_LVL[i]
        ur_in[0:h, (i - 1) * 128:i * 128] = interp_matrix(h, 128).T
        uc_in[WBASE[i]:WBASE[i] + w, :] = interp_matrix(w, 128).T

    ident = np.eye(128, dtype=np.float16)

    in_maps = []
    for core in range(NCORES):
        b, q0 = core // 4, (core % 4) * NQ
        # qh01: [128, 1200]: [half (2)] x [lvl (4) x ch (2) x n (75)]
        qh = np.zeros((128, 1200), np.float32)
        for half in range(2):
            for i in range(4):
                for c in range(2):
                    qh[:, half * 600 + i * 150 + c * 75:
                       half * 600 + i * 150 + (c + 1) * 75] = \
                        qhat[i, c, b, q0:q0 + NQ, half * 128:(half + 1) * 128].T
        in_maps.append({
            "keyT": keyTs[b],
            "qhatT": qh.astype(np.float16),
            "bias_in": biases,
            "ur_in": ur_in.astype(np.float16),
            "uc_in": uc_in.astype(np.float16),
            "ident_in": ident,
        })
    return in_maps, signs, float(bf[0])


def kernel(query, key, Wq, Wk, Wl, bl, Wf, bf, hw_lvl=None, trace=False, reps=1):
    in_maps, signs, bf_val = _host_prep(query, key, Wq, Wk, Wl, bl, Wf, bf)
    b0z = bool(np.allclose(np.asarray(bl, np.float32), 0.0))
    nc = _build_program(signs, bf_val, reps=reps, b0_zero=b0z)
    res = run_bass_kernel_spmd(nc, in_maps, list(range(NCORES)), trace=trace)
    out = np.zeros((B, N, 128 * 128, 1), np.float32)
    for core in range(NCORES):
        b, q0 = core // 4, (core % 4) * NQ
        out[b, q0:q0 + NQ, :, 0] = res.results[core]["out"].astype(np.float32)
    kernel.last_results = res
    return out


kernel.last_results = None
